# revision 5
# baseline (speedup 1.0000x reference)
"""Trainium2 Bass kernel for nn_Attention_83597243449567.

Data-parallel over batch across 8 NeuronCores: each core processes 8 of the
64 batches end-to-end (QKV proj -> nonstandard attention -> out proj); no
collectives. Weights are replicated; host pre-transposes them once so the
contraction dim lands on SBUF partitions. Matmuls run in float32r (~13
mantissa bits, full PE rate at N>=256).

Reference semantics reproduced exactly:
  qkv = x @ w_qkv.T -> q,k,v [B,H,N,D]
  attn = q @ k (contracts q's feature dim with k's token dim; D == N)
  attn = attn.swapaxes(-2,-1); P = softmax(attn, -1)
  out = (P @ v).swapaxes(1,2).reshape(B,N,C) @ w_proj.T + b_proj
"""

import sys

if "/opt/trn_rl_repo" not in sys.path:
    sys.path.insert(0, "/opt/trn_rl_repo")

import numpy as np

import concourse.bass as bass
import concourse.tile as tile
from concourse import bacc, mybir
from concourse import bass_utils
from concourse.bass import ts
from concourse.masks import make_identity

# Problem shapes (hardcoded per contract)
B, N, C = 64, 256, 2048
H, D = 8, 256
NCORES = 8
BL = B // NCORES            # batches per core
T = BL * N                  # tokens per core = 2048
F32 = mybir.dt.float32
F32R = mybir.dt.float32r

_cached = {}


def build_nc():
    if "nc" in _cached:
        return _cached["nc"]

    nc = bacc.Bacc("TRN2", target_bir_lowering=False, debug=False,
                   enable_asserts=False)

    x_d = nc.dram_tensor("x", [T, C], F32, kind="ExternalInput").ap()
    wqkvT_d = nc.dram_tensor("wqkvT", [C, 3 * C], F32R, kind="ExternalInput").ap()
    wprojT_d = nc.dram_tensor("wprojT", [C, C], F32R, kind="ExternalInput").ap()
    bproj_d = nc.dram_tensor("bproj", [C], F32, kind="ExternalInput").ap()
    y_d = nc.dram_tensor("y", [T, C], F32, kind="ExternalOutput").ap()

    TC = T // 128    # 16 token chunks
    CC = C // 128    # 16 contraction chunks
    CH = CC // 2     # weight-stream half

    with tile.TileContext(nc) as tc:
        with (
            tc.tile_pool(name="dram", bufs=1, space="DRAM") as dram,
            tc.tile_pool(name="const", bufs=1) as const_pool,
        ):
            qT_dram = dram.tile([C, T], F32R)        # q output, feature-major
            kv_dram = dram.tile([T, 2 * C], F32R)    # k|v output, token-major
            aoT_dram = dram.tile([C, T], F32R)       # attention out, feature-major

            ident = const_pool.tile([128, 128], F32)
            make_identity(nc, ident[:])
            ones_f = const_pool.tile([128, 128], F32)
            nc.gpsimd.memset(ones_f[:], 1.0)
            ones = const_pool.tile([128, 128], F32R)
            nc.scalar.copy(ones[:], ones_f[:])

            # ---------------- Phase A: x -> xT (resident, f32r) -------------
            with tc.tile_pool(name="xt", bufs=1) as xt_pool:
                xT = xt_pool.tile([128, CC, T], F32R)
                with (
                    tc.tile_pool(name="pha", bufs=4) as a_sb,
                    tc.tile_pool(name="pha_ps", bufs=4, space="PSUM") as a_ps,
                ):
                    for tci in range(TC):
                        xin = a_sb.tile([128, C], F32)
                        nc.sync.dma_start(xin[:], x_d[ts(tci, 128), :])
                        for cc in range(CC):
                            ps = a_ps.tile([128, 128], F32)
                            nc.tensor.transpose(ps[:], xin[:, ts(cc, 128)], ident[:])
                            nc.scalar.copy(xT[:, cc, ts(tci, 128)], ps[:])

                # ------------- Phase B: QKV projection -----------------------
                # q part: qT[f, t] = sum_c wqkvT[c, f] * xT[c, t]
                with tc.tile_pool(name="phb_ps", bufs=4, space="PSUM") as b_ps:
                  with (
                    tc.tile_pool(name="wq", bufs=3) as wq_pool,
                    tc.tile_pool(name="qstage", bufs=4) as qst_pool,
                  ):
                    for fc in range(CC):
                        wq_h = []
                        for h2 in range(2):
                            wt = wq_pool.tile([128, CH, 128], F32R, tag="wq")
                            nc.sync.dma_start(
                                wt[:],
                                wqkvT_d[h2 * (C // 2):(h2 + 1) * (C // 2),
                                        ts(fc, 128)]
                                .rearrange("(co p) f -> p co f", p=128),
                            )
                            wq_h.append(wt)
                        for tb in range(T // 512):
                            ps = b_ps.tile([128, 512], F32)
                            for cc in range(CC):
                                nc.tensor.matmul(
                                    ps[:], wq_h[cc // CH][:, cc % CH, :],
                                    xT[:, cc, ts(tb, 512)],
                                    start=(cc == 0), stop=(cc == CC - 1),
                                )
                            st = qst_pool.tile([128, 512], F32R)
                            nc.scalar.copy(st[:], ps[:])
                            nc.sync.dma_start(
                                qT_dram[ts(fc, 128), ts(tb, 512)], st[:])

                  # k|v part: kv[t, f] = sum_c xT[c, t] * wqkvT[c, C + f]
                  with (
                        tc.tile_pool(name="wkv", bufs=2) as wkv_pool,
                        tc.tile_pool(name="kvstage", bufs=4) as kvst_pool,
                  ):
                        for fb in range(2 * C // 512):
                            wkv_h = []
                            for h2 in range(2):
                                wt = wkv_pool.tile([128, CH, 512], F32R, tag="wkv")
                                nc.sync.dma_start(
                                    wt[:],
                                    wqkvT_d[h2 * (C // 2):(h2 + 1) * (C // 2),
                                            C + fb * 512: C + (fb + 1) * 512]
                                    .rearrange("(co p) f -> p co f", p=128),
                                )
                                wkv_h.append(wt)
                            for tci in range(TC):
                                ps = b_ps.tile([128, 512], F32)
                                for cc in range(CC):
                                    nc.tensor.matmul(
                                        ps[:], xT[:, cc, ts(tci, 128)],
                                        wkv_h[cc // CH][:, cc % CH, :],
                                        start=(cc == 0), stop=(cc == CC - 1),
                                    )
                                st = kvst_pool.tile([128, 512], F32R)
                                nc.vector.tensor_copy(st[:], ps[:])
                                nc.sync.dma_start(
                                    kv_dram[ts(tci, 128), ts(fb, 512)], st[:])

            # ---------------- Phase C: attention per (batch, head) ----------
            # S_nat[i,j] = attnT (for stats); S2[j,i] = attn.
            # P_norm[j,i] = exp(S2[j,i] - w[i]), w = rowmax + ln(rowsum of
            # exp(S_nat - rowmax)); -w folded into the S2 matmul as a K=1
            # accumulation row. outT[e,i] = v.T @ P_norm.
            with (
                tc.tile_pool(name="attn_in", bufs=3) as ain,
                tc.tile_pool(name="attn_pt", bufs=3) as apt,
                tc.tile_pool(name="attn_y", bufs=4) as ay,
                tc.tile_pool(name="attn_st", bufs=3) as ast,
                tc.tile_pool(name="ps_sn", bufs=2, space="PSUM") as ps_sn,
                tc.tile_pool(name="ps_s2", bufs=2, space="PSUM") as ps_s2,
                tc.tile_pool(name="ps_o", bufs=2, space="PSUM") as ps_o,
                tc.tile_pool(name="ps_w", bufs=2, space="PSUM") as ps_w,
            ):
                for b in range(BL):
                    for h in range(H):
                        qT_sb = ain.tile([128, 2, 256], F32R, tag="q")
                        nc.sync.dma_start(
                            qT_sb[:],
                            qT_dram[h * 256:(h + 1) * 256,
                                    b * 256:(b + 1) * 256]
                            .rearrange("(c p) t -> p c t", p=128))
                        k_sb = ain.tile([128, 2, 256], F32R, tag="k")
                        nc.sync.dma_start(
                            k_sb[:],
                            kv_dram[b * 256:(b + 1) * 256,
                                    h * 256:(h + 1) * 256]
                            .rearrange("(c p) f -> p c f", p=128))
                        v_sb = ain.tile([128, 2, 256], F32R, tag="v")
                        nc.sync.dma_start(
                            v_sb[:],
                            kv_dram[b * 256:(b + 1) * 256,
                                    C + h * 256: C + (h + 1) * 256]
                            .rearrange("(c p) f -> p c f", p=128))

                        # stats in natural orientation: w[i] = m[i] + ln Z[i]
                        negw_ps = ps_w.tile([1, 256], F32, tag="wps")
                        for ic in range(2):
                            sn = ps_sn.tile([128, 256], F32, tag="sn")
                            for dc in range(2):
                                nc.tensor.matmul(
                                    sn[:], k_sb[:, dc, ts(ic, 128)],
                                    qT_sb[:, dc, :],
                                    start=(dc == 0), stop=(dc == 1),
                                )
                            m = ast.tile([128, 1], F32, tag="m")
                            nc.vector.tensor_reduce(
                                out=m[:], in_=sn[:],
                                axis=mybir.AxisListType.X,
                                op=mybir.AluOpType.max)
                            negm = ast.tile([128, 1], F32, tag="negm")
                            nc.vector.tensor_scalar_mul(negm[:], m[:], -1.0)
                            scratch = ast.tile([128, 256], F32, tag="scratch")
                            zc = ast.tile([128, 1], F32, tag="zc")
                            nc.scalar.activation(
                                scratch[:], sn[:],
                                mybir.ActivationFunctionType.Exp,
                                bias=negm[:], accum_out=zc[:])
                            lnz = ast.tile([128, 1], F32, tag="lnz")
                            nc.scalar.activation(
                                lnz[:], zc[:],
                                mybir.ActivationFunctionType.Ln)
                            negw = ast.tile([128, 1], F32, tag="negw")
                            nc.vector.tensor_sub(negw[:], negm[:], lnz[:])
                            nc.tensor.transpose(
                                negw_ps[0:1, ts(ic, 128)], negw[:], ident[:])
                        negw_row = ast.tile([1, 256], F32R, tag="negwr")
                        nc.scalar.copy(negw_row[:], negw_ps[:])

                        # P_norm[j,i] via fused bias row
                        PT = apt.tile([128, 2, 256], F32R, tag="pt")
                        for jc in range(2):
                            s2 = ps_s2.tile([128, 256], F32, tag="s2")
                            for dc in range(2):
                                nc.tensor.matmul(
                                    s2[:], qT_sb[:, dc, ts(jc, 128)],
                                    k_sb[:, dc, :],
                                    start=(dc == 0), stop=False,
                                )
                            nc.tensor.matmul(
                                s2[:], ones[0:1, :], negw_row[:],
                                start=False, stop=True)
                            nc.scalar.activation(
                                PT[:, jc, :], s2[:],
                                mybir.ActivationFunctionType.Exp)

                        # outT[e, i] = sum_j v[j, e] * P_norm[j, i]
                        for ec in range(2):
                            ot = ps_o.tile([128, 256], F32, tag="ot")
                            for jc in range(2):
                                nc.tensor.matmul(
                                    ot[:], v_sb[:, jc, ts(ec, 128)],
                                    PT[:, jc, :],
                                    start=(jc == 0), stop=(jc == 1),
                                )
                            y_sb = ay.tile([128, 256], F32R, tag="y")
                            nc.vector.tensor_copy(y_sb[:], ot[:])
                            nc.sync.dma_start(
                                aoT_dram[h * 256 + ec * 128:
                                         h * 256 + (ec + 1) * 128,
                                         b * 256:(b + 1) * 256],
                                y_sb[:])

            # ---------------- Phase D: output projection --------------------
            with (
                tc.tile_pool(name="wp", bufs=3) as wp_pool,
                tc.tile_pool(name="ao", bufs=3) as ao_pool,
                tc.tile_pool(name="bias", bufs=2) as bias_pool,
                tc.tile_pool(name="yout", bufs=4) as y_pool,
                tc.tile_pool(name="phd_ps", bufs=4, space="PSUM") as d_ps,
            ):
                for gb in range(C // 512):
                    wp_h = []
                    for h2 in range(2):
                        wt = wp_pool.tile([128, CH, 512], F32R, tag="wp")
                        nc.sync.dma_start(
                            wt[:],
                            wprojT_d[h2 * (C // 2):(h2 + 1) * (C // 2),
                                     ts(gb, 512)]
                            .rearrange("(co p) g -> p co g", p=128))
                        wp_h.append(wt)
                    bias_sb = bias_pool.tile([128, 512], F32, tag="bias")
                    nc.sync.dma_start(
                        bias_sb[:],
                        bproj_d[None, ts(gb, 512)].to_broadcast((128, 512)))
                    for tci in range(TC):
                        ao_sb = ao_pool.tile([128, CC, 128], F32R, tag="ao")
                        nc.sync.dma_start(
                            ao_sb[:],
                            aoT_dram[:, ts(tci, 128)]
                            .rearrange("(co p) t -> p co t", p=128))
                        ps = d_ps.tile([128, 512], F32)
                        for ec in range(CC):
                            nc.tensor.matmul(
                                ps[:], ao_sb[:, ec, :], wp_h[ec // CH][:, ec % CH, :],
                                start=(ec == 0), stop=(ec == CC - 1),
                            )
                        yt = y_pool.tile([128, 512], F32, tag="yt")
                        nc.vector.tensor_add(yt[:], ps[:], bias_sb[:])
                        nc.sync.dma_start(y_d[ts(tci, 128), ts(gb, 512)], yt[:])

    nc.compile()
    _cached["nc"] = nc
    return nc


def kernel(x, w_qkv, w_proj, b_proj):
    x = np.ascontiguousarray(np.asarray(x, dtype=np.float32))
    wqkvT = np.ascontiguousarray(np.asarray(w_qkv, dtype=np.float32).T)
    wprojT = np.ascontiguousarray(np.asarray(w_proj, dtype=np.float32).T)
    b_proj = np.ascontiguousarray(np.asarray(b_proj, dtype=np.float32))

    nc = build_nc()
    in_maps = []
    for i in range(NCORES):
        xs = np.ascontiguousarray(
            x[i * BL:(i + 1) * BL].reshape(T, C))
        in_maps.append({"x": xs, "wqkvT": wqkvT, "wprojT": wprojT,
                        "bproj": b_proj})

    res = bass_utils.run_bass_kernel_spmd(nc, in_maps, core_ids=list(range(NCORES)))
    out = np.empty((B, N, C), dtype=np.float32)
    for i in range(NCORES):
        out[i * BL:(i + 1) * BL] = res.results[i]["y"].reshape(BL, N, C)
    return out


if __name__ == "__main__":
    from reference import setup_inputs, reference

    inputs = {k: np.asarray(v) for k, v in setup_inputs().items()}
    expected = np.asarray(reference(**inputs))
    actual = kernel(**inputs)
    rel = np.linalg.norm(actual - expected) / np.linalg.norm(expected)
    print("Relative error:", rel)
